# revision 42
# baseline (speedup 1.0000x reference)
"""Bass/Tile TRN2 kernel for nn_LAN_4320737100678 (dense transformer block).

Data-parallel over the batch axis across 8 NeuronCores (4 batches/core).
All activations are kept feature-major ([E, L] per batch) so that every
BatchNorm reduction and the softmax run along the free axis, and the BN
affine+ELU applications are single activation-engine passes with
per-partition scale/bias. The five BatchNorm moment sets are globalized
with four tiny in-kernel AllReduces (BN2+BN3 share one round).

Performance notes:
- All matmuls run as float32r (single-pass PE, ~4x fp32) with producer
  outputs tagged f32r for the BIR verifier.
- BN moments come from accum_out side-sums of the bias-add pass plus one
  Square pass (no bn_stats passes).
- ELU uses elu(y) = max(a, a+y), a = min(exp(y),1)-1: one ACT pass and
  four cheap ALU passes split across DVE and Pool.
- Post-allreduce affine math is vectorized over feature chunks.
- The sliding-window "unfold" (W=5) is never materialized: stage 1 is
  computed as 5 shifted matmuls accumulating into PSUM from a
  host-zero-padded copy of m1^T.
- Stage-5's l transposes are issued before the BN4 allreduce to fill the
  collective stall; lstd/out tiles reuse the l/k2 pools.
"""

import os
import sys

sys.path.insert(0, "/opt/trn_rl_repo")

import numpy as np

import concourse.bass as bass
import concourse.tile as tile
from concourse import mybir
from concourse.bass_utils import run_bass_kernel_spmd
from concourse.masks import make_identity
from concourse.vector_clock import ScopedClock

N_CORES = 8
B, L, E, W = 32, 512, 512, 5
S = W // 2
P = 128
KC = E // P            # feature chunks of 128
B_LOC = B // N_CORES   # batches per core
EPS = 1e-3
F32 = mybir.dt.float32
F32R = mybir.dt.float32r
AF = mybir.ActivationFunctionType
ALU = mybir.AluOpType
AX = mybir.AxisListType

# gpack column base offsets (each vector packed as [P, KC])
_G1, _B1, _G2, _B2, _G3, _B3, _G4, _B4, _G5, _B5 = (i * KC for i in range(10))

# 1/(N_CORES * B_LOC * L): converts a local sum into this core's share of
# the global mean so a plain AllReduce-add yields the global moments.
_MSCALE = 1.0 / (N_CORES * B_LOC * L)

_MAX_CTRL_WAITS = 1


def _split_waits(nc, max_waits=_MAX_CTRL_WAITS):
    """walrus in this container encodes at most one sync-wait slot per
    instruction. Hoist extra waits onto same-engine NOPs inserted right
    before the owning instruction (same engine => executes first)."""
    for fn in nc.m.functions:
        for bb in fn.blocks:
            rebuilt = []
            changed = False
            for ins in bb.instructions:
                si = ins.sync_info
                if si is not None and len(si.on_wait) > max_waits:
                    waits = list(si.on_wait)
                    rest = waits[max_waits:]
                    for j in range(0, len(rest), max_waits):
                        nop = mybir.InstNoOp(
                            name=f"{ins.name}_wsplit{j}",
                            engine=ins.engine,
                            bass_nofuse=True,
                            sync_info=mybir.SyncInfo(
                                on_wait=rest[j : j + max_waits], on_update=[]
                            ),
                        )
                        rebuilt.append(nop)
                    ins.sync_info = mybir.SyncInfo(
                        on_wait=waits[:max_waits], on_update=list(si.on_update)
                    )
                    changed = True
                rebuilt.append(ins)
            if changed:
                bb.instructions = rebuilt


_CACHE = {}


def _build():
    if "nc" in _CACHE:
        return _CACHE["nc"]
    nc = bass.Bass("TRN2", target_bir_lowering=False, debug=False, num_devices=N_CORES)

    m1t_d = nc.dram_tensor("m1t", [B_LOC, E, L + 2 * S], F32, kind="ExternalInput")
    f_d = nc.dram_tensor("f", [W * E, E], F32, kind="ExternalInput")
    wq_d = nc.dram_tensor("wq", [E, E], F32, kind="ExternalInput")
    wk_d = nc.dram_tensor("wk", [E, E], F32, kind="ExternalInput")
    qbt_d = nc.dram_tensor("qbt", [E, L], F32, kind="ExternalInput")
    kbt_d = nc.dram_tensor("kbt", [E, L], F32, kind="ExternalInput")
    wbt_d = nc.dram_tensor("wbt", [L, L], F32, kind="ExternalInput")
    gp_d = nc.dram_tensor("gpack", [P, 10 * KC], F32, kind="ExternalInput")
    out_d = nc.dram_tensor("outt", [B_LOC, E, L], F32, kind="ExternalOutput")

    groups = [list(range(N_CORES))]

    from contextlib import ExitStack

    with tile.TileContext(nc) as tc:
        with (
            tc.tile_pool(name="const", bufs=1) as const,
            tc.tile_pool(name="aff", bufs=4) as affp,
            tc.tile_pool(name="sums", bufs=4) as sumsp,
            tc.tile_pool(name="packs", bufs=8) as packp,
            tc.tile_pool(name="scr", bufs=16) as scr,
            tc.tile_pool(name="junk", bufs=2) as junkp,
            tc.tile_pool(name="elu", bufs=2) as elup,
            tc.tile_pool(name="psum", bufs=4, space="PSUM") as psum,
            tc.tile_pool(name="psumT", bufs=4, space="PSUM") as psumT,
            tc.tile_pool(name="dram", bufs=8, space="DRAM") as dram,
        ):
            def mmr(ps, lhsT, rhs, start, stop):
                """fp32 matmul issued as float32r: single-pass PE (1 cycle/row
                at free>=256) instead of fp32's two half-speed passes. Producers
                of both operands must write f32r-tagged outputs (BIR verifier)."""
                nc.tensor.matmul(
                    ps,
                    lhsT.bitcast(F32R),
                    rhs.bitcast(F32R),
                    start=start,
                    stop=stop,
                )

            es_l = ExitStack()
            lp = es_l.enter_context(tc.tile_pool(name="l", bufs=B_LOC * KC))

            # ---------------- Stage 1 input DMAs first -------------------
            # Interleave f and b=0 m1 tiles so the first accumulation group
            # can start as soon as its first operands land.
            es_s1 = ExitStack()
            fp = es_s1.enter_context(tc.tile_pool(name="f", bufs=W * KC))
            mp = es_s1.enter_context(tc.tile_pool(name="m1", bufs=B_LOC * KC))
            kb1p = es_s1.enter_context(tc.tile_pool(name="kb1", bufs=KC))
            f_sb, m1_sb = {}, {}

            def load_f(w, kc):
                t = fp.tile([P, E], F32, tag="f")
                r0 = (w * KC + kc) * P
                nc.sync.dma_start(
                    out=t[:].bitcast(F32R), in_=f_d[r0 : r0 + P, :].bitcast(F32R)
                )
                f_sb[w, kc] = t

            def load_m1(b, kc):
                t = mp.tile([P, L + 2 * S], F32, tag="m1")
                nc.sync.dma_start(
                    out=t[:].bitcast(F32R),
                    in_=m1t_d[b, kc * P : (kc + 1) * P, :].bitcast(F32R),
                )
                m1_sb[b, kc] = t

            for kc in range(KC):
                load_f(0, kc)
                load_m1(0, kc)
            for w in range(1, W):
                for kc in range(KC):
                    load_f(w, kc)
            for b in range(1, B_LOC):
                for kc in range(KC):
                    load_m1(b, kc)

            # kbt for stage 1 (scoped with the stage-1 pools; reloaded later
            # for stage 3 so the 16KB is freed before the stage-4/5 peak).
            kbt1_sb = {}
            for c in range(KC):
                t = kb1p.tile([P, L], F32, tag="kb1")
                nc.sync.dma_start(out=t[:], in_=kbt_d[c * P : (c + 1) * P, :])
                kbt1_sb[c] = t
            gp = const.tile([P, 10 * KC, 1], F32, tag="gp")
            nc.sync.dma_start(out=gp[:], in_=gp_d[:])
            ident = const.tile([P, P], F32, tag="ident")
            make_identity(nc, ident[:])
            epst = const.tile([P, 1], F32, tag="eps")
            nc.vector.memset(epst[:], EPS)

            def allreduce(pack, width):
                cc_in = dram.tile([P, width], F32, tag="cc")
                cc_out = dram.tile([P, width], F32, tag="cc")
                nc.gpsimd.dma_start(out=cc_in[:], in_=pack[:])
                nc.gpsimd.collective_compute(
                    "AllReduce",
                    ALU.add,
                    replica_groups=groups,
                    ins=[cc_in.opt()],
                    outs=[cc_out.opt()],
                )
                g = packp.tile([P, width // 2, 2], F32, tag="g")
                nc.gpsimd.dma_start(out=g[:], in_=cc_out[:])
                return g

            def fold_pack(zsum, zsq, n):
                """[P, n, B_LOC] side-sums -> [P, n, 2] pack of this core's
                share of the global (mean, E[x^2])."""
                sm = scr.tile([P, n, 1], F32, tag="fold")
                nc.vector.tensor_reduce(out=sm[:], in_=zsum[:], axis=AX.X, op=ALU.add)
                sq = scr.tile([P, n, 1], F32, tag="fold")
                nc.vector.tensor_reduce(out=sq[:], in_=zsq[:], axis=AX.X, op=ALU.add)
                pack = packp.tile([P, n, 2], F32, tag="pk")
                nc.vector.tensor_scalar(
                    out=pack[:, :, 0:1], in0=sm[:], scalar1=_MSCALE, scalar2=None,
                    op0=ALU.mult,
                )
                nc.vector.tensor_scalar(
                    out=pack[:, :, 1:2], in0=sq[:], scalar1=_MSCALE, scalar2=None,
                    op0=ALU.mult,
                )
                return pack

            def affines(g, gcol, bcol, n, c0=0):
                """From allreduced g [P, *, 2] (mean, E[x^2]) columns
                c0..c0+n compute scale = gamma*rsqrt(var+eps),
                bias = beta - mean*scale, vectorized over the n chunks."""
                mean = g[:, c0 : c0 + n, 0:1]
                ex2 = g[:, c0 : c0 + n, 1:2]
                sq = scr.tile([P, n, 1], F32, tag="aff_s")
                nc.vector.tensor_tensor(out=sq[:], in0=mean, in1=mean, op=ALU.mult)
                var = scr.tile([P, n, 1], F32, tag="aff_s")
                nc.vector.tensor_tensor(out=var[:], in0=ex2, in1=sq[:], op=ALU.subtract)
                sd = scr.tile([P, n, 1], F32, tag="aff_s")
                nc.scalar.activation(out=sd[:], in_=var[:], func=AF.Sqrt, bias=epst[:])
                rinv = scr.tile([P, n, 1], F32, tag="aff_s")
                nc.vector.reciprocal(rinv[:], sd[:])
                sc = affp.tile([P, n, 1], F32, tag="aff")
                nc.vector.tensor_tensor(
                    out=sc[:], in0=rinv[:], in1=gp[:, gcol : gcol + n, :], op=ALU.mult
                )
                tb = scr.tile([P, n, 1], F32, tag="aff_s")
                nc.vector.tensor_tensor(out=tb[:], in0=mean, in1=sc[:], op=ALU.mult)
                bi = affp.tile([P, n, 1], F32, tag="aff")
                nc.vector.tensor_tensor(
                    out=bi[:], in0=gp[:, bcol : bcol + n, :], in1=tb[:], op=ALU.subtract
                )
                return sc, bi

            def z_finish(ps, zt, bias_sb, zsum, zsq, col, b, round_out=True):
                """zt = ps + bias (one DVE pass, accum -> sum z); one ACT
                Square pass accumulates sum z^2. Replaces bn_stats."""
                out_ap = zt[:].bitcast(F32R) if round_out else zt[:]
                nc.vector.scalar_tensor_tensor(
                    out=out_ap, in0=ps[:], scalar=0.0, in1=bias_sb[:],
                    op0=ALU.add, op1=ALU.add, accum_out=zsum[:, col, b : b + 1],
                )
                junk = junkp.tile([P, L], F32, tag="junk")
                nc.scalar.activation(
                    out=junk[:], in_=zt[:], func=AF.Square,
                    accum_out=zsq[:, col, b : b + 1],
                )

            def elu_apply(zt, sc, bi, round_out=True):
                """zt <- elu(y), y = sc*zt + bi, via elu(y) = max(a, a+y),
                a = min(exp(y),1)-1. One ACT pass; ALU passes split over
                DVE and Pool."""
                e = elup.tile([P, L], F32, tag="elu_e")
                nc.scalar.activation(out=e[:], in_=zt[:], func=AF.Exp, bias=bi, scale=sc)
                y = elup.tile([P, L], F32, tag="elu_y")
                nc.gpsimd.tensor_scalar(
                    out=y[:], in0=zt[:], scalar1=sc, scalar2=bi,
                    op0=ALU.mult, op1=ALU.add,
                )
                a = elup.tile([P, L], F32, tag="elu_a")
                nc.vector.tensor_scalar(
                    out=a[:], in0=e[:], scalar1=1.0, scalar2=1.0,
                    op0=ALU.min, op1=ALU.subtract,
                )
                t = elup.tile([P, L], F32, tag="elu_t")
                nc.gpsimd.tensor_tensor(out=t[:], in0=a[:], in1=y[:], op=ALU.add)
                out_ap = zt[:].bitcast(F32R) if round_out else zt[:]
                nc.vector.tensor_tensor(out=out_ap, in0=a[:], in1=t[:], op=ALU.max)

            # ---------------- Stage 1: z1 = unfold(m1) @ f + kb ----------------
            l_sb = {}
            zsum1 = sumsp.tile([P, KC, B_LOC], F32, tag="sm")
            zsq1 = sumsp.tile([P, KC, B_LOC], F32, tag="sm")
            for b in range(B_LOC):
                for mc in range(KC):
                    ps = psum.tile([P, L], F32, tag="ps")
                    n = 0
                    for w in range(W):
                        for kc in range(KC):
                            mmr(
                                ps[:],
                                f_sb[w, kc][:, mc * P : (mc + 1) * P],
                                m1_sb[b, kc][:, w : w + L],
                                start=(n == 0),
                                stop=(n == W * KC - 1),
                            )
                            n += 1
                    zt = lp.tile([P, L], F32, tag="l")
                    z_finish(ps, zt, kbt1_sb[mc], zsum1, zsq1, mc, b)
                    l_sb[b, mc] = zt

            es_s1.close()

            pack1 = fold_pack(zsum1, zsq1, KC)
            g1 = allreduce(pack1, KC * 2)
            sc1, bi1 = affines(g1, _G1, _B1, KC)
            for b in range(B_LOC):
                for mc in range(KC):
                    elu_apply(l_sb[b, mc], sc1[:, mc, :], bi1[:, mc, :])

            # ------------- Stage 2/3: q2 = l@wq + qb, k2 = l@wk + kb -------------
            es_z = ExitStack()
            z2p = es_z.enter_context(tc.tile_pool(name="z2", bufs=B_LOC * KC))
            z3p = es_z.enter_context(tc.tile_pool(name="z3", bufs=B_LOC * KC))
            es_wqk = ExitStack()
            qkbp = es_wqk.enter_context(tc.tile_pool(name="qkb", bufs=2 * KC))
            wqkp = es_wqk.enter_context(tc.tile_pool(name="wqk", bufs=2 * KC))
            qbt_sb, kbt_sb = {}, {}
            wq_sb, wk_sb = {}, {}
            for kc in range(KC):
                t = wqkp.tile([P, E], F32, tag="wqk")
                nc.sync.dma_start(
                    out=t[:].bitcast(F32R),
                    in_=wq_d[kc * P : (kc + 1) * P, :].bitcast(F32R),
                )
                wq_sb[kc] = t
                t = wqkp.tile([P, E], F32, tag="wqk")
                nc.sync.dma_start(
                    out=t[:].bitcast(F32R),
                    in_=wk_d[kc * P : (kc + 1) * P, :].bitcast(F32R),
                )
                wk_sb[kc] = t
                t = qkbp.tile([P, L], F32, tag="qkb")
                nc.sync.dma_start(out=t[:], in_=qbt_d[kc * P : (kc + 1) * P, :])
                qbt_sb[kc] = t
                t = qkbp.tile([P, L], F32, tag="qkb")
                nc.sync.dma_start(out=t[:], in_=kbt_d[kc * P : (kc + 1) * P, :])
                kbt_sb[kc] = t

            zsum23 = sumsp.tile([P, 2 * KC, B_LOC], F32, tag="sm")
            zsq23 = sumsp.tile([P, 2 * KC, B_LOC], F32, tag="sm")
            q2_sb, k2_sb = {}, {}
            for b in range(B_LOC):
                for mc in range(KC):
                    ps = psum.tile([P, L], F32, tag="ps")
                    for kc in range(KC):
                        mmr(
                            ps[:],
                            wq_sb[kc][:, mc * P : (mc + 1) * P],
                            l_sb[b, kc][:],
                            start=(kc == 0),
                            stop=(kc == KC - 1),
                        )
                    zt = z2p.tile([P, L], F32, tag="z2")
                    z_finish(ps, zt, qbt_sb[mc], zsum23, zsq23, mc, b)
                    q2_sb[b, mc] = zt

                    ps = psum.tile([P, L], F32, tag="ps")
                    for kc in range(KC):
                        mmr(
                            ps[:],
                            wk_sb[kc][:, mc * P : (mc + 1) * P],
                            l_sb[b, kc][:],
                            start=(kc == 0),
                            stop=(kc == KC - 1),
                        )
                    zt = z3p.tile([P, L], F32, tag="z3")
                    z_finish(ps, zt, kbt_sb[mc], zsum23, zsq23, KC + mc, b)
                    k2_sb[b, mc] = zt

            pack23 = fold_pack(zsum23, zsq23, 2 * KC)
            g23 = allreduce(pack23, 4 * KC)
            sc2, bi2 = affines(g23, _G2, _B2, KC, c0=0)
            sc3, bi3 = affines(g23, _G3, _B3, KC, c0=KC)

            for b in range(B_LOC):
                for mc in range(KC):
                    elu_apply(q2_sb[b, mc], sc2[:, mc, :], bi2[:, mc, :])
                    elu_apply(k2_sb[b, mc], sc3[:, mc, :], bi3[:, mc, :])

            # ------------- Stage 4a: wT = (q2 @ k2^T)^T + wb^T -------------
            es_wqk.close()
            es_w45 = ExitStack()
            lstdp = es_w45.enter_context(tc.tile_pool(name="lstd", bufs=B_LOC * KC))
            # wraw and the softmax output share one ring: each wt tile reuses
            # the wraw buffer that its own exp pass just consumed.
            wrawtp = es_w45.enter_context(
                tc.tile_pool(name="wrawt", bufs=B_LOC * KC)
            )
            wbtp = es_w45.enter_context(tc.tile_pool(name="wbt", bufs=KC))
            wbt_sb = {}
            for c in range(KC):
                t = wbtp.tile([P, L], F32, tag="wbt")
                nc.sync.dma_start(out=t[:], in_=wbt_d[c * P : (c + 1) * P, :])
                wbt_sb[c] = t

            zsum4 = sumsp.tile([P, KC, B_LOC], F32, tag="sm")
            zsq4 = sumsp.tile([P, KC, B_LOC], F32, tag="sm")
            wraw_sb = {}
            for b in range(B_LOC):
                for kc in range(KC):
                    ps = psum.tile([P, L], F32, tag="ps")
                    for ec in range(KC):
                        mmr(
                            ps[:],
                            k2_sb[b, ec][:, kc * P : (kc + 1) * P],
                            q2_sb[b, ec][:],
                            start=(ec == 0),
                            stop=(ec == KC - 1),
                        )
                    wt = wrawtp.tile([P, L], F32, tag="wrawt")
                    z_finish(ps, wt, wbt_sb[kc], zsum4, zsq4, kc, b)
                    wraw_sb[b, kc] = wt

            # ---- stage-5 l transposes: issued here to fill the BN4 stall ----
            lstd_sb = {}
            for b in range(B_LOC):
                for kc in range(KC):
                    pst_t = {}
                    for mc in range(KC):
                        pst = psumT.tile([P, P], F32, tag="psT")
                        nc.tensor.transpose(
                            pst[:], l_sb[b, mc][:, kc * P : (kc + 1) * P], ident[:]
                        )
                        pst_t[mc] = pst
                    lst = lstdp.tile([P, E], F32, tag="lstd")
                    for mc in range(KC):
                        nc.vector.tensor_copy(
                            lst[:, mc * P : (mc + 1) * P].bitcast(F32R), pst_t[mc][:]
                        )
                    lstd_sb[b, kc] = lst

            pack4 = fold_pack(zsum4, zsq4, KC)
            g4 = allreduce(pack4, KC * 2)
            sc4, bi4 = affines(g4, _G4, _B4, KC)

            # ---------------- Stage 4b: BN4 + softmax over q ----------------
            # softmax(bn4(w)) over the free axis, with the BN affine fused into
            # the exp (sc4/bi4 are constant along the softmax axis; softmax is
            # shift-invariant and post-BN values are unit-scale, so no max
            # subtraction is needed for fp32 range safety).
            wt_sb = {}
            for b in range(B_LOC):
                for kc in range(KC):
                    raw = wraw_sb[b, kc]
                    e = elup.tile([P, L], F32, tag="elu_e")
                    ssum = scr.tile([P, 1], F32, tag="scr")
                    nc.scalar.activation(
                        out=e[:], in_=raw[:], func=AF.Exp,
                        bias=bi4[:, kc, :], scale=sc4[:, kc, :], accum_out=ssum[:],
                    )
                    rs = scr.tile([P, 1], F32, tag="scr")
                    nc.vector.reciprocal(rs[:], ssum[:])
                    t = wrawtp.tile([P, L], F32, tag="wrawt")
                    nc.vector.tensor_scalar_mul(t[:].bitcast(F32R), e[:], rs[:])
                    wt_sb[b, kc] = t

            # ---------------- Stage 5: out = w @ l, BN5 + ELU ----------------
            # out tiles reuse the k2 (z3) pool ring.
            zsum5 = sumsp.tile([P, KC, B_LOC], F32, tag="sm")
            zsq5 = sumsp.tile([P, KC, B_LOC], F32, tag="sm")
            out_sb = {}
            for b in range(B_LOC):
                for mc in range(KC):
                    ps = psum.tile([P, L], F32, tag="ps")
                    for kc in range(KC):
                        mmr(
                            ps[:],
                            lstd_sb[b, kc][:, mc * P : (mc + 1) * P],
                            wt_sb[b, kc][:],
                            start=(kc == 0),
                            stop=(kc == KC - 1),
                        )
                    ot = z3p.tile([P, L], F32, tag="z3")
                    nc.vector.tensor_scalar(
                        out=ot[:], in0=ps[:], scalar1=0.0, scalar2=0.0,
                        op0=ALU.add, op1=ALU.add,
                        accum_out=zsum5[:, mc, b : b + 1],
                    )
                    junk = junkp.tile([P, L], F32, tag="junk")
                    nc.scalar.activation(
                        out=junk[:], in_=ot[:], func=AF.Square,
                        accum_out=zsq5[:, mc, b : b + 1],
                    )
                    out_sb[b, mc] = ot

            pack5 = fold_pack(zsum5, zsq5, KC)
            g5 = allreduce(pack5, KC * 2)
            sc5, bi5 = affines(g5, _G5, _B5, KC)
            for b in range(B_LOC):
                for mc in range(KC):
                    elu_apply(out_sb[b, mc], sc5[:, mc, :], bi5[:, mc, :], round_out=False)
                    nc.sync.dma_start(
                        out=out_d[b, mc * P : (mc + 1) * P, :], in_=out_sb[b, mc][:]
                    )

            es_w45.close()
            es_z.close()
            es_l.close()

    _split_waits(nc)
    _CACHE["nc"] = nc
    return nc


def _pack_affine(vecs):
    cols = []
    for v in vecs:
        cols.append(np.ascontiguousarray(np.asarray(v, np.float32).reshape(KC, P).T))
    return np.ascontiguousarray(np.concatenate(cols, axis=1))


def kernel(m1, f, wq, wk, qb, kb, wb, g1, b1, g2, b2, g3, b3, g4, b4, g5, b5):
    m1 = np.asarray(m1, np.float32)
    nc = _build()
    # host-side zero pad along L so the kernel needs no memsets
    m1t = np.zeros((B, E, L + 2 * S), np.float32)
    m1t[:, :, S : S + L] = m1.transpose(0, 2, 1)
    f_h = np.ascontiguousarray(np.asarray(f, np.float32))
    wq_h = np.ascontiguousarray(np.asarray(wq, np.float32))
    wk_h = np.ascontiguousarray(np.asarray(wk, np.float32))
    qbt = np.ascontiguousarray(np.asarray(qb, np.float32).T)
    kbt = np.ascontiguousarray(np.asarray(kb, np.float32).T)
    wbt = np.ascontiguousarray(np.asarray(wb, np.float32).T)
    gpack = _pack_affine([g1, b1, g2, b2, g3, b3, g4, b4, g5, b5])

    shared = {
        "f": f_h, "wq": wq_h, "wk": wk_h,
        "qbt": qbt, "kbt": kbt, "wbt": wbt, "gpack": gpack,
    }
    in_maps = [
        {"m1t": np.ascontiguousarray(m1t[i * B_LOC : (i + 1) * B_LOC]), **shared}
        for i in range(N_CORES)
    ]
    trace = os.environ.get("KERNEL_TRACE") == "1"
    res = run_bass_kernel_spmd(nc, in_maps, list(range(N_CORES)), trace=trace)
    _CACHE["last_results"] = res

    out = np.empty((B, L, E), np.float32)
    for i in range(N_CORES):
        out[i * B_LOC : (i + 1) * B_LOC] = res.results[i]["outt"].transpose(0, 2, 1)
    return out
